# revision 13
# baseline (speedup 1.0000x reference)
"""Trainium2 Bass kernel for nn_Encoder_31550829756513 (2-layer dual-branch GCN).

Strategy (8 NeuronCores, node-partitioned graph parallel):
  - Host: build sym-norms for both branches, append self-loop pseudo-edges,
    sort edges by (destination block, source), pack per-(core, slot, range)
    128-edge columns with a shared compile-time column schedule. Source
    indices are split into 32768-row ranges so they fit dma_gather's int16
    index format.
  - Device, per core (single SPMD program, all offsets static):
      phase B: full hw = x @ W1 table computed locally on every core
          (x is a full input; avoids an AllGather entirely)
      L1: per (group, range): one batched dma_gather (row i -> partition
          i%128, column i//128); per column: two fused selector ops
          (iota==dst)*norm split across DVE and GpSimd, two PE matmuls
          accumulate z_g|z_p in PSUM; per block: h1 = relu(z + b1), then
          premultiply by W2 on-device (PE transpose + matmul) so only the
          O-wide table t = h1 @ W2 [NPAD, 2*O] fp16 is AllGathered.
      AllGather t -> full fp16 table [NPAD, 2*O]
      L2: same message pass on t; per block: logits via dot with dense_w,
          softmax-of-2 == sigmoid(lg-lp), blend in O-space, + b2 -> out.
  - Host: concatenate output shards, slice to N rows.
"""

import os
import numpy as np

P = 128
R15 = 32768          # dma_gather int16 index range
NR = 4               # number of index ranges covering NPAD
PREP_VER = 5

_FP16 = np.float16


class Cfg:
    def __init__(self, n, e, d=256, h=128, o=64, ncores=8, gb=7, gg=5):
        self.N = n
        self.E = e
        self.D = d
        self.H = h
        self.O = o
        self.ncores = ncores
        self.NBLK = -(-n // P)
        self.NB = -(-self.NBLK // ncores)
        self.CORE_ROWS = self.NB * P
        self.NPAD = ncores * self.CORE_ROWS
        self.NBLK_ALL = self.NPAD // P
        self.GB = gb
        self.GG = gg


FULL = Cfg(100000, 1600000)


def _schedule(cfg, C):
    """Build the shared compile-time column schedule from C[slot][range].

    Returns (groups, passes, block_cols, NCH):
      groups: [(g0, gs)] slot groups
      passes: per group: list of (r, col_start, ncols) dma_gather passes
      block_cols: per slot: ordered list of absolute column ids
      NCH: total columns
    """
    NB, GG = cfg.NB, cfg.GG
    groups = [(g0, min(GG, NB - g0)) for g0 in range(0, NB, GG)]
    passes = []
    block_cols = [[] for _ in range(NB)]
    col = 0
    for (g0, gs) in groups:
        gp = []
        for r in range(NR):
            start = col
            for s in range(g0, g0 + gs):
                for _ in range(C[s][r]):
                    block_cols[s].append(col)
                    col += 1
            if col > start:
                gp.append((r, start, col - start))
        passes.append(gp)
    return groups, passes, block_cols, col


# ----------------------------------------------------------------------------
# Host preprocessing
# ----------------------------------------------------------------------------

def _preprocess(cfg, x, edge_index, ppmi_edge_weight, W1, b1, W2, b2,
                dense_w, dense_b):
    n, e = cfg.N, cfg.E
    row = np.asarray(edge_index[0], dtype=np.int64).astype(np.int32)
    col = np.asarray(edge_index[1], dtype=np.int64).astype(np.int32)
    ppmi = np.asarray(ppmi_edge_weight, dtype=np.float64)

    sl = np.arange(n, dtype=np.int32)
    row_sl = np.concatenate([row, sl])
    ones_n = np.ones(n, dtype=np.float64)

    def sym_dis(ew):
        deg = np.bincount(row_sl, weights=ew, minlength=n)
        return np.where(deg > 0, deg ** -0.5, 0.0)

    dis_g = sym_dis(np.concatenate([np.ones(e), ones_n]))
    dis_p = sym_dis(np.concatenate([ppmi, ones_n]))

    # augmented edge list: real edges + self-loop pseudo-edges
    src_a = np.concatenate([row, sl])
    dst_a = np.concatenate([col, sl])
    gn_a = np.concatenate([dis_g[row] * dis_g[col],
                           (dis_g * dis_g)]).astype(np.float32)
    pn_a = np.concatenate([dis_p[row] * ppmi * dis_p[col],
                           (dis_p * dis_p)]).astype(np.float32)

    blk_all = dst_a >> 7
    order = np.lexsort((src_a, blk_all))  # dst block, then ascending src
    src_s = src_a[order]
    dst_s = dst_a[order]
    gn_s = gn_a[order]
    pn_s = pn_a[order]
    blk = blk_all[order]
    rng_s = (src_s >> 15).astype(np.int64)
    dstloc = (dst_s & 127).astype(np.float32)

    core_of = blk // cfg.NB
    slot_of = blk - core_of * cfg.NB

    # per (core, slot, range) counts -> shared schedule C[slot][range]
    key = (core_of * cfg.NB + slot_of) * NR + rng_s
    cnt = np.bincount(key, minlength=cfg.ncores * cfg.NB * NR).reshape(
        cfg.ncores, cfg.NB, NR)
    C = np.ceil(cnt.max(axis=0) / P).astype(np.int64)  # [NB, NR]
    empty = C.sum(axis=1) == 0
    C[empty, 0] = 1

    groups, passes, block_cols, NCH = _schedule(cfg, C)

    # absolute column start per (slot, range), following the schedule order
    colstart = np.zeros((cfg.NB, NR), dtype=np.int64)
    for s in range(cfg.NB):
        pos = 0
        for r in range(NR):
            colstart[s][r] = -1
    # reconstruct from block_cols: columns of slot s are ordered by range
    for s in range(cfg.NB):
        pos = 0
        for r in range(NR):
            if C[s][r] > 0:
                colstart[s][r] = block_cols[s][pos]
                pos += C[s][r]

    # rank of each edge within its (core, slot, range) segment
    seg_key = key * cfg.ncores + core_of  # unique per (core, slot, range)
    seg_key = (core_of * cfg.NB + slot_of) * NR + rng_s
    seg_key_full = seg_key  # edges sorted by (blk, src) => segments contiguous
    seg_starts = np.zeros(cfg.ncores * cfg.NB * NR, dtype=np.int64)
    np.cumsum(cnt.ravel()[:-1], out=seg_starts[1:])
    # order is sorted by (blk, src); within a block, ranges ascending since
    # src ascending. Edge position within segment:
    first_of_seg = seg_starts[seg_key_full]
    # need positions in the sorted edge array: segments are contiguous runs
    # because sort key (blk, src) groups (core,slot) then range.
    pos_in_arr = np.arange(src_s.shape[0], dtype=np.int64)
    # start of each run in sorted order: compute via change points
    chg = np.empty(src_s.shape[0], dtype=bool)
    chg[0] = True
    chg[1:] = seg_key_full[1:] != seg_key_full[:-1]
    run_start = np.maximum.accumulate(np.where(chg, pos_in_arr, 0))
    rank = pos_in_arr - run_start

    lane = (rank & 127).astype(np.int64)
    kcol = rank >> 7
    abscol = colstart[slot_of, rng_s] + kcol
    c_arr = core_of.astype(np.int64)

    dst_stream = np.zeros((cfg.ncores, P, NCH), dtype=np.float32)
    nrm_stream = np.zeros((cfg.ncores, P, NCH, 2), dtype=np.float32)
    dst_stream[c_arr, lane, abscol] = dstloc
    nrm_stream[c_arr, lane, abscol, 0] = gn_s
    nrm_stream[c_arr, lane, abscol, 1] = pn_s

    # int16 index stream in dma_gather wrap layout.
    # Within pass (g, r): flat i = (abscol - pass_start)*128 + lane.
    # idx16[:, pass_start*8 + i//16][i % 16] = src - r*R15 (replicated x8).
    idx16 = np.zeros((cfg.ncores, 16, NCH * 8), dtype=np.int16)
    # map abscol -> its pass start
    pass_start_of_col = np.zeros(NCH, dtype=np.int64)
    for gp in passes:
        for (r, start, ncols) in gp:
            pass_start_of_col[start:start + ncols] = start
    ps_e = pass_start_of_col[abscol]
    i_flat = (abscol - ps_e) * P + lane
    idx16[c_arr, i_flat % 16, ps_e * 8 + i_flat // 16] = \
        (src_s - rng_s * R15).astype(np.int16)
    idx16_full = np.ascontiguousarray(
        np.tile(idx16, (1, 8, 1)))  # replicate to 128 partitions

    xT = np.zeros((cfg.D, cfg.NPAD), dtype=_FP16)
    xT[:, :n] = np.asarray(x, dtype=np.float32).T.astype(_FP16)

    W1f = np.asarray(W1, dtype=np.float32).astype(_FP16)
    W2f = np.asarray(W2, dtype=np.float32).astype(_FP16)
    b1r2 = np.tile(np.asarray(b1, dtype=np.float32)[None, :], (P, 2))
    dwb = np.tile(np.asarray(dense_w, dtype=np.float32).ravel()[None, :],
                  (P, 1))
    b2r = np.tile(np.asarray(b2, dtype=np.float32)[None, :], (P, 1))

    in_maps = []
    for c in range(cfg.ncores):
        in_maps.append({
            "xT": xT,
            "w1": W1f, "w2": W2f, "b1r2": b1r2, "dwb": dwb, "b2r": b2r,
            "idx16": idx16_full[c], "dsts": dst_stream[c],
            "nrms": nrm_stream[c],
        })
    Ctup = tuple(tuple(int(v) for v in row_) for row_ in C)
    return in_maps, Ctup


# ----------------------------------------------------------------------------
# Device program
# ----------------------------------------------------------------------------

def build_program(cfg, Ctup):
    from concourse import bass, mybir, tile, bacc
    from concourse.masks import make_identity

    dt16 = mybir.dt.float16
    dt32 = mybir.dt.float32
    AOT = mybir.AluOpType
    AFT = mybir.ActivationFunctionType

    C = [list(row_) for row_ in Ctup]
    groups, passes, block_cols, NCH = _schedule(cfg, C)
    NB, H, O, D = cfg.NB, cfg.H, cfg.O, cfg.D
    O2 = 2 * O

    nc = bacc.Bacc("TRN2", debug=False, enable_asserts=False,
                   num_devices=cfg.ncores)

    xT = nc.dram_tensor("xT", [D, cfg.NPAD], dt16, kind="ExternalInput")
    w1 = nc.dram_tensor("w1", [D, H], dt16, kind="ExternalInput")
    w2 = nc.dram_tensor("w2", [H, O], dt16, kind="ExternalInput")
    b1r2 = nc.dram_tensor("b1r2", [P, 2 * H], dt32, kind="ExternalInput")
    dwb = nc.dram_tensor("dwb", [P, O], dt32, kind="ExternalInput")
    b2r = nc.dram_tensor("b2r", [P, O], dt32, kind="ExternalInput")
    idx16 = nc.dram_tensor("idx16", [P, NCH * 8], mybir.dt.int16,
                           kind="ExternalInput")
    dsts = nc.dram_tensor("dsts", [P, NCH], dt32, kind="ExternalInput")
    nrms = nc.dram_tensor("nrms", [P, NCH, 2], dt32, kind="ExternalInput")
    outp = nc.dram_tensor("out", [cfg.CORE_ROWS, O], dt32,
                          kind="ExternalOutput")

    hw_full = nc.dram_tensor("hw_full", [cfg.NPAD, H], dt16)
    t_shard = nc.dram_tensor("t_shard", [cfg.CORE_ROWS, O2], dt16)
    t_full = nc.dram_tensor("t_full", [cfg.NPAD, O2], dt16,
                            addr_space="Shared")

    groups_all = [list(range(cfg.ncores))]
    seln = [0]

    with tile.TileContext(nc) as tc:
        with tc.tile_pool(name="const", bufs=1) as cpool:
            w1a = cpool.tile([P, H], dt16)
            w1b = cpool.tile([P, H], dt16)
            nc.sync.dma_start(out=w1a[:], in_=w1[0:P, :])
            nc.sync.dma_start(out=w1b[:], in_=w1[P:2 * P, :])
            w2sb = cpool.tile([P, O], dt16)
            nc.sync.dma_start(out=w2sb[:], in_=w2[:, :])
            b1sb = cpool.tile([P, 2 * H], dt32)
            nc.sync.dma_start(out=b1sb[:], in_=b1r2[:, :])
            dwsb = cpool.tile([P, O], dt32)
            nc.sync.dma_start(out=dwsb[:], in_=dwb[:, :])
            b2sb = cpool.tile([P, O], dt32)
            nc.sync.dma_start(out=b2sb[:], in_=b2r[:, :])
            it16 = cpool.tile([P, P], mybir.dt.int16)
            nc.gpsimd.iota(it16[:], pattern=[[1, P]], base=0,
                           channel_multiplier=0)
            iotaf = cpool.tile([P, P], dt16)
            nc.vector.tensor_copy(out=iotaf[:], in_=it16[:])
            ident16 = cpool.tile([P, P], dt16)
            make_identity(nc, ident16[:])
            idx_sb = cpool.tile([P, NCH * 8], mybir.dt.int16)
            nc.sync.dma_start(out=idx_sb[:], in_=idx16[:, :])
            dst_sb = cpool.tile([P, NCH], dt32)
            nc.sync.dma_start(out=dst_sb[:], in_=dsts[:, :])
            nrm_sb = cpool.tile([P, NCH, 2], dt32)
            nc.sync.dma_start(out=nrm_sb[:], in_=nrms[:, :, :])

            def build_sel(spool, c, br):
                sb = spool.tile([P, P], dt16, tag="sel")
                eng = nc.gpsimd if (seln[0] % 4 == 3) else nc.vector
                seln[0] += 1
                eng.tensor_scalar(
                    out=sb[:], in0=iotaf[:],
                    scalar1=dst_sb[:, c:c + 1],
                    scalar2=nrm_sb[:, c, br:br + 1],
                    op0=AOT.is_equal, op1=AOT.mult)
                return sb

            def gather_group(gpool, gi, table, tbl_rows, tag):
                (g0, gs) = groups[gi]
                gp = passes[gi]
                base = gp[0][1]
                kg = sum(ncols for (_, _, ncols) in gp)
                gth = gpool.tile([P, kg, P], dt16, tag=tag)
                for (r, start, ncols) in gp:
                    lo = r * R15
                    hi = min(lo + R15, tbl_rows)
                    nc.gpsimd.dma_gather(
                        out_ap=gth[:, start - base:start - base + ncols, :],
                        in_ap=table.ap()[lo:hi, :],
                        idxs_ap=idx_sb[:, start * 8:(start + ncols) * 8],
                        num_idxs=ncols * P,
                        num_idxs_reg=ncols * P,
                        elem_size=P,
                    )
                return gth, base

            # ------------- phase B: full hw = x @ W1 on every core ----------
            hw_view = hw_full.ap().rearrange("(t p) h -> p t h", p=P)
            with tc.tile_pool(name="phB", bufs=3) as xp, \
                 tc.tile_pool(name="phBp", bufs=4, space="PSUM") as bp, \
                 tc.tile_pool(name="phBh", bufs=2) as hp:
                for g0 in range(0, cfg.NBLK_ALL, cfg.GB):
                    gs = min(cfg.GB, cfg.NBLK_ALL - g0)
                    xa = xp.tile([P, gs * P], dt16, tag="xa")
                    xb = xp.tile([P, gs * P], dt16, tag="xb")
                    nc.sync.dma_start(out=xa[:],
                                      in_=xT[0:P, g0 * P:(g0 + gs) * P])
                    nc.sync.dma_start(out=xb[:],
                                      in_=xT[P:2 * P, g0 * P:(g0 + gs) * P])
                    hwg = hp.tile([P, gs, H], dt16, tag="hwg")
                    for t in range(gs):
                        ps = bp.tile([P, H], dt32, tag="bps")
                        nc.tensor.matmul(out=ps[:],
                                         lhsT=xa[:, t * P:(t + 1) * P],
                                         rhs=w1a[:], start=True, stop=False)
                        nc.tensor.matmul(out=ps[:],
                                         lhsT=xb[:, t * P:(t + 1) * P],
                                         rhs=w1b[:], start=False, stop=True)
                        if t % 2 == 0:
                            nc.scalar.activation(out=hwg[:, t, :], in_=ps[:],
                                                 func=AFT.Copy)
                        else:
                            nc.vector.tensor_copy(out=hwg[:, t, :], in_=ps[:])
                    nc.sync.dma_start(out=hw_view[:, g0:g0 + gs, :], in_=hwg[:])

            # ------------- L1 message pass + on-device @W2 ------------------
            t_view = t_shard.ap().rearrange("(t p) o -> p t o", p=P)
            with tc.tile_pool(name="d_g", bufs=3) as gpool, \
                 tc.tile_pool(name="d_z", bufs=3, space="PSUM") as zpool, \
                 tc.tile_pool(name="d_s", bufs=48) as spool, \
                 tc.tile_pool(name="d_h", bufs=3) as hpool, \
                 tc.tile_pool(name="d_tp", bufs=2, space="PSUM") as tppool, \
                 tc.tile_pool(name="d_tm", bufs=2, space="PSUM") as tmpool, \
                 tc.tile_pool(name="d_o", bufs=2) as opool:
                for gi, (g0, gs) in enumerate(groups):
                    gth, base = gather_group(gpool, gi, hw_full, cfg.NPAD,
                                             "gth1")
                    og = opool.tile([P, gs, O2], dt16, tag="og1")
                    for s in range(gs):
                        blk = g0 + s
                        cols = block_cols[blk]
                        z = zpool.tile([P, 2 * H], dt32, tag="z")
                        for br in range(2):
                            for j, c in enumerate(cols):
                                sb = build_sel(spool, c, br)
                                nc.tensor.matmul(
                                    out=z[:, br * H:(br + 1) * H],
                                    lhsT=sb[:], rhs=gth[:, c - base, :],
                                    start=(j == 0),
                                    stop=(j == len(cols) - 1))
                        hb = hpool.tile([P, 2, H], dt16, tag="hb")
                        nc.vector.tensor_tensor(
                            out=hb[:].rearrange("p b h -> p (b h)"),
                            in0=z[:], in1=b1sb[:], op=AOT.add)
                        hr = hpool.tile([P, 2, H], dt16, tag="hr")
                        nc.scalar.activation(
                            out=hr[:].rearrange("p b h -> p (b h)"),
                            in_=hb[:].rearrange("p b h -> p (b h)"),
                            func=AFT.Relu)
                        tp = tppool.tile([P, 2, P], dt16, tag="tp")
                        nc.tensor.transpose(out=tp[:, 0, :], in_=hr[:, 0, :],
                                            identity=ident16[:])
                        nc.tensor.transpose(out=tp[:, 1, :], in_=hr[:, 1, :],
                                            identity=ident16[:])
                        hT = hpool.tile([P, 2, P], dt16, tag="hT")
                        nc.scalar.activation(
                            out=hT[:].rearrange("p b q -> p (b q)"),
                            in_=tp[:].rearrange("p b q -> p (b q)"),
                            func=AFT.Copy)
                        tm = tmpool.tile([P, 2, O], dt32, tag="tm")
                        nc.tensor.matmul(out=tm[:, 0, :], lhsT=hT[:, 0, :],
                                         rhs=w2sb[:], start=True, stop=True)
                        nc.tensor.matmul(out=tm[:, 1, :], lhsT=hT[:, 1, :],
                                         rhs=w2sb[:], start=True, stop=True)
                        nc.scalar.activation(
                            out=og[:, s, :],
                            in_=tm[:].rearrange("p b o -> p (b o)"),
                            func=AFT.Copy)
                    nc.sync.dma_start(
                        out=t_view[:, g0:g0 + gs, :], in_=og[:])

            nc.gpsimd.collective_compute(
                "AllGather", AOT.bypass, replica_groups=groups_all,
                ins=[t_shard.ap().opt()], outs=[t_full.ap().opt()])

            # ------------- L2 message pass + combine ------------------------
            out_view = outp.ap().rearrange("(t p) o -> p t o", p=P)
            with tc.tile_pool(name="f_g", bufs=3) as gpool, \
                 tc.tile_pool(name="f_z", bufs=3, space="PSUM") as zpool, \
                 tc.tile_pool(name="f_s", bufs=48) as spool, \
                 tc.tile_pool(name="f_e", bufs=4) as epool, \
                 tc.tile_pool(name="f_o", bufs=2) as opool:
                for gi, (g0, gs) in enumerate(groups):
                    gth, base = gather_group(gpool, gi, t_full, cfg.NPAD,
                                             "gth2")
                    og = opool.tile([P, gs, O], dt32, tag="og2")
                    for s in range(gs):
                        blk = g0 + s
                        cols = block_cols[blk]
                        zo = zpool.tile([P, O2], dt32, tag="zo")
                        for br in range(2):
                            for j, c in enumerate(cols):
                                sb = build_sel(spool, c, br)
                                nc.tensor.matmul(
                                    out=zo[:, br * O:(br + 1) * O],
                                    lhsT=sb[:],
                                    rhs=gth[:, c - base,
                                            br * O:(br + 1) * O],
                                    start=(j == 0),
                                    stop=(j == len(cols) - 1))
                        zps = epool.tile([P, O], dt32, tag="zps")
                        nc.scalar.activation(out=zps[:], in_=zo[:, O:O2],
                                             func=AFT.Copy)
                        zd = epool.tile([P, O], dt32, tag="zd")
                        nc.vector.tensor_tensor(out=zd[:], in0=zo[:, 0:O],
                                                in1=zps[:],
                                                op=AOT.subtract)
                        scr = epool.tile([P, O], dt32, tag="scr")
                        dl = epool.tile([P, 1], dt32, tag="dl")
                        nc.vector.tensor_tensor(out=scr[:], in0=zd[:],
                                                in1=dwsb[:], op=AOT.mult)
                        nc.vector.tensor_reduce(out=dl[:], in_=scr[:],
                                                axis=mybir.AxisListType.X,
                                                op=AOT.add)
                        wg = epool.tile([P, 1], dt32, tag="wg")
                        nc.scalar.activation(out=wg[:], in_=dl[:],
                                             func=AFT.Sigmoid)
                        bl = epool.tile([P, O], dt32, tag="bl")
                        nc.vector.scalar_tensor_tensor(
                            out=bl[:], in0=zd[:], scalar=wg[:],
                            in1=zps[:], op0=AOT.mult, op1=AOT.add)
                        nc.vector.tensor_tensor(out=og[:, s, :], in0=bl[:],
                                                in1=b2sb[:], op=AOT.add)
                    nc.sync.dma_start(out=out_view[:, g0:g0 + gs, :],
                                      in_=og[:])

    nc.compile()
    return nc


_BUILD_CACHE = {}


def _get_program(cfg, Ctup):
    key = (cfg.N, cfg.E, cfg.GG, Ctup)
    if key not in _BUILD_CACHE:
        _BUILD_CACHE[key] = build_program(cfg, Ctup)
    return _BUILD_CACHE[key]


LAST_RESULTS = None


def _run(cfg, inputs):
    from concourse.bass_utils import run_bass_kernel_spmd
    global LAST_RESULTS
    in_maps, Ctup = _preprocess(cfg, **inputs)
    nc = _get_program(cfg, Ctup)
    trace = bool(int(os.environ.get("KERNEL_TRACE", "0")))
    res = run_bass_kernel_spmd(nc, in_maps, core_ids=list(range(cfg.ncores)),
                               trace=trace)
    LAST_RESULTS = res
    out = np.concatenate([res.results[c]["out"] for c in range(cfg.ncores)],
                         axis=0)[:cfg.N]
    return np.ascontiguousarray(out.astype(np.float32))


def kernel(x, edge_index, ppmi_edge_weight, W1, b1, W2, b2, dense_w, dense_b):
    return _run(FULL, dict(x=x, edge_index=edge_index,
                           ppmi_edge_weight=ppmi_edge_weight, W1=W1, b1=b1,
                           W2=W2, b2=b2, dense_w=dense_w, dense_b=dense_b))
